# revision 27
# baseline (speedup 1.0000x reference)
"""Distributed multi-head attention block on 8 TRN2 NeuronCores.

Reference computation (B=2, S=2048, D=1024, H=16, DH=64):
    q = split_heads(q_ @ Wq + bq); k = ...; v = ...
    attn = softmax(q k^T / 8)  (mask is all-ones -> identity row mask)
    out = (merge_heads(attn @ v) + q_) @ Wf + bf

Sharding: 16 heads split 8 ways (2 heads / core); each core handles BOTH
batches.  The "virtual q" axis is b-major: vq = b*2048 + s (4096 total).

Per core c (heads 2c, 2c+1; d-dims 128c..128c+128):
  1. Projections (fp8 in, bf16 out): QT/KT [128 dh, 4096 vq], V [vk, 128 dh].
     Weights arrive host-pre-transposed (contiguous DMA); the x-streams are
     split per 128-row din tile, one tensor per DMA queue (sync=K,
     scalar=Q, gpsimd=V), batch-0 first, so K/Q scores + the ScalarE exp
     stream start ~tens of us before all projections are done.  Batch-1
     x tiles are single-buffered so their DMAs self-throttle behind the
     batch-0 consumers.
  2. Attention, transposed formulation: ST[k, q] = KT^T Q per (head, k-tile,
     q-chunk); exp via ScalarE straight from PSUM (scale folded in);
     OT[dh, q] = V^T P accumulated over k-tiles; row-sums via ones-matmuls;
     softmax denominators broadcast with K=1 matmuls; normalize + residual
     on VectorE -> ZT_local [128 d, 4096 vq] (fp8).  Score emission runs
     two q-chunks ahead (lookahead-2) so the exp stream never starves.
  3. Query ownership: core j owns queries [128j, 128j+128) of each
     1024-query window w (4 windows); one AllToAll per window -- the first
     three hide under attention, the last is only 128 KB.  Per-window
     unpack + residual + fc tile (128 rows) follow; the fc for the first
     windows overlaps the last AllToAll.

Host side: casts/transposes inputs (numpy), feeds per-core shards, places
each core's four [128, 1024] output chunks, adds bf.  If the mask is not
all-ones (never happens with this problem's generator), falls back to a
numpy reference implementation.
"""

import sys

sys.path.insert(0, "/opt/trn_rl_repo")

import ml_dtypes
import numpy as np

import concourse.bass as bass
import concourse.tile as tile
from concourse import bacc, mybir
from concourse.bass_utils import run_bass_kernel_spmd

B, S, D, H = 2, 2048, 1024, 16
DH = D // H  # 64
N_CORES = 8
VQ = B * S  # 4096 virtual q (b-major)
NQC = VQ // 512  # 8 q-chunks of 512
NKT = S // 128  # 16 k-tiles per batch
NDIN = D // 128  # 8 din tiles

BF16 = mybir.dt.bfloat16
FP8 = mybir.dt.float8e4
F32 = mybir.dt.float32
AF = mybir.ActivationFunctionType
ALU = mybir.AluOpType
BF16NP = ml_dtypes.bfloat16
FP8NP = ml_dtypes.float8_e4m3
WSCALE = 32.0

_CACHE = {}


def _build():
    nc = bacc.Bacc(None, target_bir_lowering=False)

    xq = nc.declare_dram_parameter("xq", [D, VQ], FP8, isOutput=False)
    xk = nc.declare_dram_parameter("xk", [D, VQ], FP8, isOutput=False)
    xv = nc.declare_dram_parameter("xv", [D, VQ], FP8, isOutput=False)
    wq = nc.declare_dram_parameter("wq", [128, NDIN * 128], FP8, isOutput=False)
    wk = nc.declare_dram_parameter("wk", [128, NDIN * 128], FP8, isOutput=False)
    wv = nc.declare_dram_parameter("wv", [128, NDIN * 128], FP8, isOutput=False)
    wf = nc.declare_dram_parameter("wf", [128, NDIN * 1024], BF16, isOutput=False)
    xresfc = nc.declare_dram_parameter("xresfc", [128, NQC * 512], BF16, isOutput=False)
    bq = nc.declare_dram_parameter("bq", [128, 1], F32, isOutput=False)
    bk = nc.declare_dram_parameter("bk", [128, 1], F32, isOutput=False)
    bv = nc.declare_dram_parameter("bv", [1, 128], BF16, isOutput=False)
    out = nc.declare_dram_parameter("out", [512, D], F32, isOutput=True)

    with tile.TileContext(nc) as tc:
        with (
            tc.tile_pool(name="persist", bufs=1) as sbp,
            tc.tile_pool(name="dram", bufs=1, space="DRAM") as dram,
        ):
            # ---- persistent SBUF tensors ----
            qt_sb = sbp.tile([128, VQ], BF16)  # [2 heads x 64 dh, vq]
            kt_sb = sbp.tile([128, VQ], BF16)  # [2 heads x 64 dh, vkey]
            v_sb = sbp.tile([128, 32 * 128], BF16)  # [k in tile, (b,kt) x 128 dh]
            wq_sb = sbp.tile([128, NDIN * 128], FP8)
            wk_sb = sbp.tile([128, NDIN * 128], FP8)
            wv_sb = sbp.tile([128, NDIN * 128], FP8)
            wf_sb = sbp.tile([128, NDIN * 1024], BF16)
            xresfc_sb = sbp.tile([128, NQC * 512], BF16)
            zt_local = sbp.tile([128, VQ], FP8)
            bq_sb = sbp.tile([128, 1], F32)
            bk_sb = sbp.tile([128, 1], F32)
            bv_sb = sbp.tile([1, 128], BF16)
            ones_col = sbp.tile([128, 1], BF16)
            ones_sb = sbp.tile([128, 128], BF16)
            nc.vector.memset(ones_col[:], 1.0)
            nc.vector.memset(ones_sb[:], 1.0)
            # load the exp table-set (~2.7us) now, not at the first real exp
            actwarm = sbp.tile([1, 1], BF16)
            nc.scalar.activation(actwarm[:], ones_col[0:1, 0:1], AF.Exp)

            # small weights first; each queue leads with its first-needed one
            nc.sync.dma_start(wk_sb[:], wk[:])
            nc.scalar.dma_start(wq_sb[:], wq[:])
            nc.gpsimd.dma_start(wv_sb[:], wv[:])
            nc.gpsimd.dma_start(bq_sb[:], bq[:])
            nc.gpsimd.dma_start(bk_sb[:], bk[:])
            nc.gpsimd.dma_start(bv_sb[:], bv[:])

            # attention-long pools (released after the qc loop)
            ptp = tc.alloc_tile_pool(name="ptp", bufs=34)
            nrm = tc.alloc_tile_pool(name="nrm", bufs=3)

            # x inputs (single-buffered: batch-1 loads self-throttle behind
            # their batch-0 consumers), split per din tile for streaming
            xvp = tc.alloc_tile_pool(name="xvp", bufs=1)
            xin = tc.alloc_tile_pool(name="xin", bufs=1)
            xk_b, xq_b, xv_b = [], [], []
            for b in range(2):
                xk_b.append(xin.tile([128, NDIN * 2048], FP8, name=f"xk{b}", tag="xk"))
                xq_b.append(xin.tile([128, NDIN * 2048], FP8, name=f"xq{b}", tag="xq"))
                xv_b.append(xvp.tile([128, NDIN * 2048], FP8, name=f"xv{b}", tag="xv"))

            def load_x(dst, src, b, eng):
                # per-din 2KB-line slices: finer slicing (512B lines) was
                # measured slower -- DMA descriptor overhead dominates
                for din in range(NDIN):
                    eng.dma_start(
                        dst[:, 2048 * din : 2048 * (din + 1)],
                        src[128 * din : 128 * (din + 1), 2048 * b : 2048 * (b + 1)],
                    )

            load_x(xk_b[0], xk, 0, nc.sync)
            load_x(xq_b[0], xq, 0, nc.scalar)
            load_x(xv_b[0], xv, 0, nc.gpsimd)
            load_x(xk_b[1], xk, 1, nc.sync)
            load_x(xq_b[1], xq, 1, nc.scalar)
            load_x(xv_b[1], xv, 1, nc.gpsimd)

            # late-needed tensors trail the x loads on the scalar queue
            nc.scalar.dma_start(xresfc_sb[:], xresfc[:])
            nc.scalar.dma_start(wf_sb[:], wf[:])


            def v_unit(pool, b, kt):
                vkt = 16 * b + kt
                vp = pool.tile([128, 512], F32, name=f"vps{vkt}", tag="ps")
                for din in range(NDIN):
                    nc.tensor.matmul(
                        vp[:, 0:128],
                        lhsT=xv_b[b][:, 2048 * din + 128 * kt : 2048 * din + 128 * (kt + 1)],
                        rhs=wv_sb[:, 128 * din : 128 * (din + 1)],
                        start=(din == 0),
                        stop=False,
                    )
                nc.tensor.matmul(
                    vp[:, 0:128], lhsT=ones_sb[0:1, :], rhs=bv_sb[:],
                    start=False, stop=True,
                )
                nc.vector.tensor_copy(v_sb[:, 128 * vkt : 128 * (vkt + 1)], vp[:, 0:128])

            def qk_proj(pool, name, xt, w_sb, b_sb, dst, b, nchunk):
                # nchunk chunks in flight (pool slots); 8 din steps each
                def chunk_unit(qcc):
                    pt_ = pool.tile([128, 512], F32, name=f"{name}c{b}_{qcc}", tag="ps")
                    for din in range(NDIN):
                        nc.tensor.matmul(
                            pt_[:],
                            lhsT=w_sb[:, 128 * din : 128 * (din + 1)],
                            rhs=xt[:, 2048 * din + 512 * qcc : 2048 * din + 512 * (qcc + 1)],
                            start=(din == 0),
                            stop=(din == NDIN - 1),
                        )
                    nc.vector.tensor_scalar_add(
                        dst[:, 512 * (4 * b + qcc) : 512 * (4 * b + qcc + 1)],
                        pt_[:],
                        b_sb[:],
                    )

                return [lambda q=q: chunk_unit(q) for q in range(4)]

            # ============ phase 1a: K+Q projections (batch 0) ============
            # din-major across BOTH tensors on all 8 banks: each din step
            # issues as soon as that slice of xk0/xq0 lands, so projections
            # finish ~2us after the last DMA slice instead of serializing
            with tc.tile_pool(name="ps1a", bufs=8, space="PSUM") as ps1a:
                warm = ps1a.tile([128, 512], F32, name="warm", tag="ps")
                for i in range(24):
                    nc.tensor.matmul(
                        warm[:, 0:128], lhsT=ones_sb[:], rhs=ones_sb[:],
                        start=True, stop=True,
                    )
                kpts = [
                    ps1a.tile([128, 512], F32, name=f"kc{qcc}", tag="ps")
                    for qcc in range(4)
                ]
                qpts = [
                    ps1a.tile([128, 512], F32, name=f"qc{qcc}", tag="ps")
                    for qcc in range(4)
                ]
                for din in range(NDIN):
                    for pts_, w_sb, xt in ((kpts, wk_sb, xk_b[0]), (qpts, wq_sb, xq_b[0])):
                        for qcc in range(4):
                            nc.tensor.matmul(
                                pts_[qcc][:],
                                lhsT=w_sb[:, 128 * din : 128 * (din + 1)],
                                rhs=xt[:, 2048 * din + 512 * qcc : 2048 * din + 512 * (qcc + 1)],
                                start=(din == 0),
                                stop=(din == NDIN - 1),
                            )
                for qcc in range(4):
                    nc.vector.tensor_scalar_add(
                        kt_sb[:, 512 * qcc : 512 * (qcc + 1)], kpts[qcc][:], bk_sb[:]
                    )
                    nc.vector.tensor_scalar_add(
                        qt_sb[:, 512 * qcc : 512 * (qcc + 1)], qpts[qcc][:], bq_sb[:]
                    )
            stp = tc.alloc_tile_pool(name="stp", bufs=2, space="PSUM")  # 4 banks

            # =================== phase 2: attention ===================
            # query ownership: core j owns queries [128j, 128j+128) of each
            # 1024-query window w (4 windows); one AllToAll per window, so
            # the first three hide under attention and the last is 128 KB.
            a2a_ins = [dram.tile([1024, 128], FP8, name=f"a2ai{w}") for w in range(4)]
            a2a_outs = [dram.tile([1024, 128], FP8, name=f"a2ao{w}") for w in range(4)]

            pjp = tc.alloc_tile_pool(name="pjp", bufs=4, space="PSUM")  # 4 banks

            def emit_qk(qc, kt):
                b = qc // 4
                q0 = 512 * qc
                kk = 2048 * b + 128 * kt
                st = stp.tile([128, 1024], F32, name=f"st{qc}_{kt}", tag="st")
                pt = ptp.tile([128, 1024], BF16, name=f"pt{qc}_{kt}", tag="pt")
                for h in range(2):
                    nc.tensor.matmul(
                        st[:, 512 * h : 512 * (h + 1)],
                        lhsT=kt_sb[64 * h : 64 * (h + 1), kk : kk + 128],
                        rhs=qt_sb[64 * h : 64 * (h + 1), q0 : q0 + 512],
                        start=True,
                        stop=True,
                    )
                nc.scalar.activation(pt[:], st[:], AF.Exp, scale=0.125 / (WSCALE * WSCALE))
                return pt

            def emit_pv(qc, kt, ot, rs, pt):
                b = qc // 4
                vk = 128 * (16 * b + kt)
                first = kt == 0
                last = kt == NKT - 1
                for h in range(2):
                    nc.tensor.matmul(
                        ot[64 * h : 64 * (h + 1), :],
                        lhsT=v_sb[:, vk + 64 * h : vk + 64 * (h + 1)],
                        rhs=pt[:, 512 * h : 512 * (h + 1)],
                        start=first,
                        stop=last,
                        tile_position=(0, 64 * h),
                    )
                for h in range(2):
                    nc.tensor.matmul(
                        rs[32 * h : 32 * h + 1, :],
                        lhsT=ones_col[:],
                        rhs=pt[:, 512 * h : 512 * (h + 1)],
                        start=first,
                        stop=last,
                        tile_position=(0, 32 * h),
                    )

            def finish(qc, ot_sb, rs_bf):
                # normalize + residual; PE part (bc) rides in the middle of
                # the next round's matmul stream
                q0 = 512 * qc
                bc = rsp.tile([128, 512], F32, name=f"bc{qc}", tag="rs")
                nc.tensor.matmul(
                    bc[0:64, :], lhsT=ones_sb[0:1, 0:64], rhs=rs_bf[0:1, :],
                    start=True, stop=True, tile_position=(0, 0),
                )
                nc.tensor.matmul(
                    bc[64:128, :], lhsT=ones_sb[32:33, 0:64], rhs=rs_bf[32:33, :],
                    start=True, stop=True, tile_position=(32, 64),
                )
                recipb = nrm.tile([128, 512], F32, name=f"recipb{qc}", tag="recipb")
                a2a_in = a2a_ins[qc // 2]
                # halves pipelined: the first two a2a blocks ship while the
                # second half's reciprocal still runs (shaves tail latency)
                for half in range(2):
                    h0 = 256 * half
                    nc.vector.reciprocal(recipb[:, h0 : h0 + 256], bc[:, h0 : h0 + 256])
                    nc.vector.tensor_tensor(
                        zt_local[:, q0 + h0 : q0 + h0 + 256],
                        ot_sb[:, h0 : h0 + 256],
                        recipb[:, h0 : h0 + 256],
                        ALU.mult,
                    )
                    for blk in (2 * half, 2 * half + 1):
                        j = 4 * (qc % 2) + blk
                        nc.sync.dma_start(
                            a2a_in[128 * j : 128 * (j + 1), :],
                            zt_local[:, q0 + 128 * blk : q0 + 128 * (blk + 1)],
                        )

            # prologue: qk(qc0) first (earliest exp), V b0, qk(qc1), then
            # all batch-1 projection units (their DMAs land later).
            ptsd = {}
            ptsd[0] = [emit_qk(0, kt) for kt in range(NKT)]
            for kt in range(NKT):
                v_unit(pjp, 0, kt)
            ptsd[1] = [emit_qk(1, kt) for kt in range(NKT)]
            for u in qk_proj(pjp, "k", xk_b[1], wk_sb, bk_sb, kt_sb, 1, 4):
                u()
            for u in qk_proj(pjp, "q", xq_b[1], wq_sb, bq_sb, qt_sb, 1, 4):
                u()
            for kt in range(NKT):
                v_unit(pjp, 1, kt)
            pjp.release()
            otp = tc.alloc_tile_pool(name="otp", bufs=2, space="PSUM")
            rsp = tc.alloc_tile_pool(name="rsp", bufs=2, space="PSUM")

            pending = None
            for qc in range(NQC):
                ot = otp.tile([128, 512], F32, name=f"ot{qc}", tag="ot")
                rs = rsp.tile([128, 512], F32, name=f"rs{qc}", tag="rs")
                pts = ptsd.pop(qc)
                nxt = []
                for kt in range(NKT):
                    emit_pv(qc, kt, ot, rs, pts[kt])
                    if qc + 2 < NQC:
                        nxt.append(emit_qk(qc + 2, kt))
                    if kt == 0 and pending is not None:
                        finish(*pending)
                        if pending[0] % 2 == 1:
                            w = pending[0] // 2
                            nc.gpsimd.collective_compute(
                                "AllToAll",
                                ALU.bypass,
                                replica_groups=[list(range(N_CORES))],
                                ins=[a2a_ins[w].opt()],
                                outs=[a2a_outs[w].opt()],
                            )
                        pending = None
                if qc + 2 < NQC:
                    ptsd[qc + 2] = nxt
                # drain psum accumulators to SBUF on VectorE so banks free
                # without PE stalls
                ot_sb = nrm.tile([128, 512], F32, name=f"otsb{qc}", tag="otsb")
                nc.vector.tensor_copy(ot_sb[:], ot[:])
                rs_bf = nrm.tile([128, 512], BF16, name=f"rsbf{qc}", tag="rsbf")
                nc.vector.tensor_copy(rs_bf[0:1, :], rs[0:1, :])
                nc.vector.tensor_copy(rs_bf[32:33, :], rs[32:33, :])
                if pending is not None:
                    finish(*pending)
                pending = (qc, ot_sb, rs_bf)
            finish(*pending)

            # last collective fires as soon as its inputs land
            nc.gpsimd.collective_compute(
                "AllToAll",
                ALU.bypass,
                replica_groups=[list(range(N_CORES))],
                ins=[a2a_ins[3].opt()],
                outs=[a2a_outs[3].opt()],
            )

            rsp.release()
            otp.release()
            stp.release()
            xin.release()
            xvp.release()

            # =================== phase 3: unpack + fc ===================
            # per-window unpack + residual + fc; first three windows' work
            # overlaps the last AllToAll.  tile_wait_until keeps the
            # scheduler from hoisting phase-3 engine work ahead of the
            # attention drains (head-of-line blocking otherwise).
            with (
                tc.tile_pool(name="fcps", bufs=4, space="PSUM") as fcps,
                tc.tile_pool(name="ysb", bufs=2) as ysb,
                tc.tile_wait_until(0.5),
            ):
                for w in range(4):
                    for t in range(NDIN):
                        nc.sync.dma_start(
                            zt_local[:, 512 * t + 128 * w : 512 * t + 128 * (w + 1)],
                            a2a_outs[w][128 * t : 128 * (t + 1), :],
                        )
                    for t in range(NDIN):
                        c0 = 512 * t + 128 * w
                        nc.vector.scalar_tensor_tensor(
                            qt_sb[:, c0 : c0 + 128],
                            zt_local[:, c0 : c0 + 128],
                            1.0 / WSCALE,
                            xresfc_sb[:, c0 : c0 + 128],
                            ALU.mult,
                            ALU.add,
                        )
                    row = 128 * w
                    y = ysb.tile([128, 1024], F32, name=f"y{w}", tag="y")
                    for nb in range(2):
                        yp = fcps.tile([128, 512], F32, name=f"yp{w}_{nb}", tag="yp")
                        for j in range(NDIN):
                            nc.tensor.matmul(
                                yp[:],
                                lhsT=qt_sb[:, 512 * j + row : 512 * j + row + 128],
                                rhs=wf_sb[:, 1024 * j + 512 * nb : 1024 * j + 512 * (nb + 1)],
                                start=(j == 0),
                                stop=(j == NDIN - 1),
                            )
                        nc.vector.tensor_copy(y[:, 512 * nb : 512 * (nb + 1)], yp[:])
                    nc.sync.dma_start(out[row : row + 128, :], y[:])

            nrm.release()
            ptp.release()

    nc.compile()
    return nc


def _numpy_reference(q_, k_, v_, mask, Wq, bq, Wk, bk, Wv, bv, Wf, bf):
    q_ = np.asarray(q_, np.float32)
    k_ = np.asarray(k_, np.float32)
    v_ = np.asarray(v_, np.float32)
    b = q_.shape[0]

    def split(x):
        return x.reshape(b, -1, H, DH).transpose(0, 2, 1, 3)

    q = split(q_ @ Wq + bq)
    k = split(k_ @ Wk + bk)
    v = split(v_ @ Wv + bv)
    attn = np.einsum("bhqd,bhkd->bhqk", q, k) / np.sqrt(np.float32(DH))
    attn = np.where(np.asarray(mask)[:, None, :, None], attn, np.float32(-1e12))
    attn = attn - attn.max(axis=-1, keepdims=True)
    e = np.exp(attn)
    p = e / e.sum(axis=-1, keepdims=True)
    o = np.einsum("bhqk,bhkd->bhqd", p, v)
    o = o.transpose(0, 2, 1, 3).reshape(b, -1, D)
    return (o + q_) @ Wf + bf


def kernel(q_, k_, v_, mask, Wq, bq, Wk, bk, Wv, bv, Wf, bf):
    mask = np.asarray(mask)
    if not mask.all():
        return _numpy_reference(q_, k_, v_, mask, Wq, bq, Wk, bk, Wv, bv, Wf, bf)

    q_ = np.asarray(q_, np.float32)
    k_ = np.asarray(k_, np.float32)
    v_ = np.asarray(v_, np.float32)

    # transposed, b-major-concatenated inputs (shared across cores)
    xq_f = np.ascontiguousarray(np.concatenate([q_[b].T for b in range(B)], axis=1))
    xq = xq_f.astype(FP8NP)
    xk = np.ascontiguousarray(np.concatenate([k_[b].T for b in range(B)], axis=1)).astype(FP8NP)
    xv = np.ascontiguousarray(np.concatenate([v_[b].T for b in range(B)], axis=1)).astype(FP8NP)
    wf_b = np.ascontiguousarray(
        np.asarray(Wf, np.float32).astype(BF16NP)
        .reshape(8, 128, 1024).transpose(1, 0, 2).reshape(128, 8192)
    )

    def sbufize(a, cols):
        # [1024, cols] -> SBUF layout [128, 8*cols] (din-tile-major columns)
        return np.ascontiguousarray(
            a.reshape(8, 128, cols).transpose(1, 0, 2).reshape(128, 8 * cols)
        )

    in_maps = []
    for c in range(N_CORES):
        d0 = 128 * c
        xres_cols = np.empty((D, 512), np.float32)
        for w in range(4):
            xres_cols[:, 128 * w : 128 * (w + 1)] = q_[w // 2].T[
                :, 1024 * (w % 2) + 128 * c : 1024 * (w % 2) + 128 * (c + 1)
            ]
        in_maps.append(
            {
                "xq": xq,
                "xk": xk,
                "xv": xv,
                "xresfc": sbufize(xres_cols.astype(BF16NP), 512),
                "wq": sbufize((np.asarray(Wq, np.float32)[:, d0 : d0 + 128] * WSCALE).astype(FP8NP), 128),
                "wk": sbufize((np.asarray(Wk, np.float32)[:, d0 : d0 + 128] * WSCALE).astype(FP8NP), 128),
                "wv": sbufize((np.asarray(Wv, np.float32)[:, d0 : d0 + 128] * WSCALE).astype(FP8NP), 128),
                "wf": wf_b,
                "bq": np.ascontiguousarray(np.asarray(bq, np.float32)[d0 : d0 + 128, None] * WSCALE),
                "bk": np.ascontiguousarray(np.asarray(bk, np.float32)[d0 : d0 + 128, None] * WSCALE),
                "bv": np.ascontiguousarray(np.asarray(bv, np.float32)[None, d0 : d0 + 128] * WSCALE).astype(BF16NP),
            }
        )

    if "nc" not in _CACHE:
        _CACHE["nc"] = _build()
    res = run_bass_kernel_spmd(_CACHE["nc"], in_maps, core_ids=list(range(N_CORES)))

    out = np.empty((B, S, D), np.float32)
    for c in range(N_CORES):
        y = res.results[c]["out"]
        for w in range(4):
            out[w // 2, 1024 * (w % 2) + 128 * c : 1024 * (w % 2) + 128 * (c + 1), :] = (
                y[128 * w : 128 * (w + 1)]
            )
    out += np.asarray(bf, np.float32)[None, None, :]
    return out


if __name__ == "__main__":
    rng = np.random.default_rng(0)
    args = dict(
        q_=rng.standard_normal((B, S, D), dtype=np.float32),
        k_=rng.standard_normal((B, S, D), dtype=np.float32),
        v_=rng.standard_normal((B, S, D), dtype=np.float32),
        mask=np.ones((B, S), bool),
        Wq=rng.standard_normal((D, D), dtype=np.float32) * 0.02,
        bq=np.zeros(D, np.float32),
        Wk=rng.standard_normal((D, D), dtype=np.float32) * 0.02,
        bk=np.zeros(D, np.float32),
        Wv=rng.standard_normal((D, D), dtype=np.float32) * 0.02,
        bv=np.zeros(D, np.float32),
        Wf=rng.standard_normal((D, D), dtype=np.float32) * 0.02,
        bf=np.zeros(D, np.float32),
    )
    got = kernel(**args)
    want = _numpy_reference(**args)
    rel = np.abs(got - want).max() / np.abs(want).max()
    print("rel_err:", rel)


# revision 29
# speedup vs baseline: 1.0134x; 1.0134x over previous
"""Distributed multi-head attention block on 8 TRN2 NeuronCores.

Reference computation (B=2, S=2048, D=1024, H=16, DH=64):
    q = split_heads(q_ @ Wq + bq); k = ...; v = ...
    attn = softmax(q k^T / 8)  (mask is all-ones -> identity row mask)
    out = (merge_heads(attn @ v) + q_) @ Wf + bf

Sharding: 16 heads split 8 ways (2 heads / core); each core handles BOTH
batches.  The "virtual q" axis is b-major: vq = b*2048 + s (4096 total).

Per core c (heads 2c, 2c+1; d-dims 128c..128c+128):
  1. Projections (fp8 in, bf16 out): QT/KT [128 dh, 4096 vq], V [vk, 128 dh].
     Weights arrive host-pre-transposed (contiguous DMA); the x-streams are
     split per 128-row din tile, one tensor per DMA queue (sync=K,
     scalar=Q, gpsimd=V), batch-0 first, so K/Q scores + the ScalarE exp
     stream start ~tens of us before all projections are done.  Batch-1
     x tiles are single-buffered so their DMAs self-throttle behind the
     batch-0 consumers.
  2. Attention, transposed formulation: ST[k, q] = KT^T Q per (head, k-tile,
     q-chunk); exp via ScalarE straight from PSUM (scale folded in);
     OT[dh, q] = V^T P accumulated over k-tiles; row-sums via ones-matmuls;
     softmax denominators broadcast with K=1 matmuls; normalize + residual
     on VectorE -> ZT_local [128 d, 4096 vq] (fp8).  Score emission runs
     two q-chunks ahead (lookahead-2) so the exp stream never starves.
  3. Query ownership: core j owns queries [128j, 128j+128) of each
     1024-query window w (4 windows); one AllToAll per window -- the first
     three hide under attention, the last is only 128 KB.  Per-window
     unpack + residual + fc tile (128 rows) follow; the fc for the first
     windows overlaps the last AllToAll.

Host side: casts/transposes inputs (numpy), feeds per-core shards, places
each core's four [128, 1024] output chunks, adds bf.  If the mask is not
all-ones (never happens with this problem's generator), falls back to a
numpy reference implementation.
"""

import sys

sys.path.insert(0, "/opt/trn_rl_repo")

import ml_dtypes
import numpy as np

import concourse.bass as bass
import concourse.tile as tile
from concourse import bacc, mybir
from concourse.bass_utils import run_bass_kernel_spmd

B, S, D, H = 2, 2048, 1024, 16
DH = D // H  # 64
N_CORES = 8
VQ = B * S  # 4096 virtual q (b-major)
NQC = VQ // 512  # 8 q-chunks of 512
NKT = S // 128  # 16 k-tiles per batch
NDIN = D // 128  # 8 din tiles

BF16 = mybir.dt.bfloat16
FP8 = mybir.dt.float8e4
F32 = mybir.dt.float32
AF = mybir.ActivationFunctionType
ALU = mybir.AluOpType
BF16NP = ml_dtypes.bfloat16
FP8NP = ml_dtypes.float8_e4m3
WSCALE = 32.0

_CACHE = {}


def _build():
    nc = bacc.Bacc(None, target_bir_lowering=False)

    xq = nc.declare_dram_parameter("xq", [D, VQ], FP8, isOutput=False)
    xk = nc.declare_dram_parameter("xk", [D, VQ], FP8, isOutput=False)
    xv = nc.declare_dram_parameter("xv", [D, VQ], FP8, isOutput=False)
    wq = nc.declare_dram_parameter("wq", [128, NDIN * 128], FP8, isOutput=False)
    wk = nc.declare_dram_parameter("wk", [128, NDIN * 128], FP8, isOutput=False)
    wv = nc.declare_dram_parameter("wv", [128, NDIN * 128], FP8, isOutput=False)
    wf = nc.declare_dram_parameter("wf", [128, NDIN * 1024], BF16, isOutput=False)
    xresfc = nc.declare_dram_parameter("xresfc", [128, NQC * 512], BF16, isOutput=False)
    bq = nc.declare_dram_parameter("bq", [128, 1], F32, isOutput=False)
    bk = nc.declare_dram_parameter("bk", [128, 1], F32, isOutput=False)
    bv = nc.declare_dram_parameter("bv", [1, 128], BF16, isOutput=False)
    out = nc.declare_dram_parameter("out", [512, D], F32, isOutput=True)

    with tile.TileContext(nc) as tc:
        with (
            tc.tile_pool(name="persist", bufs=1) as sbp,
            tc.tile_pool(name="dram", bufs=1, space="DRAM") as dram,
        ):
            # ---- persistent SBUF tensors ----
            qt_sb = sbp.tile([128, VQ], BF16)  # [2 heads x 64 dh, vq]
            kt_sb = sbp.tile([128, VQ], BF16)  # [2 heads x 64 dh, vkey]
            v_sb = sbp.tile([128, 32 * 128], BF16)  # [k in tile, (b,kt) x 128 dh]
            wq_sb = sbp.tile([128, NDIN * 128], FP8)
            wk_sb = sbp.tile([128, NDIN * 128], FP8)
            wv_sb = sbp.tile([128, NDIN * 128], FP8)
            wf_sb = sbp.tile([128, NDIN * 1024], BF16)
            xresfc_sb = sbp.tile([128, NQC * 512], BF16)
            zt_local = sbp.tile([128, VQ], FP8)
            bq_sb = sbp.tile([128, 1], F32)
            bk_sb = sbp.tile([128, 1], F32)
            bv_sb = sbp.tile([1, 128], BF16)
            ones_col = sbp.tile([128, 1], BF16)
            ones_sb = sbp.tile([128, 128], BF16)
            nc.vector.memset(ones_col[:], 1.0)
            nc.vector.memset(ones_sb[:], 1.0)

            # small weights first; each queue leads with its first-needed one
            nc.sync.dma_start(wk_sb[:], wk[:])
            nc.scalar.dma_start(wq_sb[:], wq[:])
            nc.gpsimd.dma_start(wv_sb[:], wv[:])
            nc.gpsimd.dma_start(bq_sb[:], bq[:])
            nc.gpsimd.dma_start(bk_sb[:], bk[:])
            nc.gpsimd.dma_start(bv_sb[:], bv[:])

            # attention-long pools (released after the qc loop)
            stp = tc.alloc_tile_pool(name="stp", bufs=2, space="PSUM")  # 4 banks
            ptp = tc.alloc_tile_pool(name="ptp", bufs=34)
            nrm = tc.alloc_tile_pool(name="nrm", bufs=3)

            # x inputs (single-buffered: batch-1 loads self-throttle behind
            # their batch-0 consumers), split per din tile for streaming
            xvp = tc.alloc_tile_pool(name="xvp", bufs=1)
            xin = tc.alloc_tile_pool(name="xin", bufs=1)
            xk_b, xq_b, xv_b = [], [], []
            for b in range(2):
                xk_b.append(xin.tile([128, NDIN * 2048], FP8, name=f"xk{b}", tag="xk"))
                xq_b.append(xin.tile([128, NDIN * 2048], FP8, name=f"xq{b}", tag="xq"))
                xv_b.append(xvp.tile([128, NDIN * 2048], FP8, name=f"xv{b}", tag="xv"))

            def load_x(dst, src, b, eng):
                # per-din 2KB-line slices: finer slicing (512B lines) was
                # measured slower -- DMA descriptor overhead dominates
                for din in range(NDIN):
                    eng.dma_start(
                        dst[:, 2048 * din : 2048 * (din + 1)],
                        src[128 * din : 128 * (din + 1), 2048 * b : 2048 * (b + 1)],
                    )

            load_x(xk_b[0], xk, 0, nc.sync)
            load_x(xq_b[0], xq, 0, nc.scalar)
            load_x(xv_b[0], xv, 0, nc.gpsimd)
            load_x(xk_b[1], xk, 1, nc.sync)
            load_x(xq_b[1], xq, 1, nc.scalar)
            load_x(xv_b[1], xv, 1, nc.gpsimd)

            # late-needed tensors trail the x loads on the scalar queue
            nc.scalar.dma_start(xresfc_sb[:], xresfc[:])
            nc.scalar.dma_start(wf_sb[:], wf[:])


            def v_unit(pool, b, kt):
                vkt = 16 * b + kt
                vp = pool.tile([128, 512], F32, name=f"vps{vkt}", tag="ps")
                for din in range(NDIN):
                    nc.tensor.matmul(
                        vp[:, 0:128],
                        lhsT=xv_b[b][:, 2048 * din + 128 * kt : 2048 * din + 128 * (kt + 1)],
                        rhs=wv_sb[:, 128 * din : 128 * (din + 1)],
                        start=(din == 0),
                        stop=False,
                    )
                nc.tensor.matmul(
                    vp[:, 0:128], lhsT=ones_sb[0:1, :], rhs=bv_sb[:],
                    start=False, stop=True,
                )
                nc.vector.tensor_copy(v_sb[:, 128 * vkt : 128 * (vkt + 1)], vp[:, 0:128])

            def qk_proj(pool, name, xt, w_sb, b_sb, dst, b, nchunk):
                # nchunk chunks in flight (pool slots); 8 din steps each
                def chunk_unit(qcc):
                    pt_ = pool.tile([128, 512], F32, name=f"{name}c{b}_{qcc}", tag="ps")
                    for din in range(NDIN):
                        nc.tensor.matmul(
                            pt_[:],
                            lhsT=w_sb[:, 128 * din : 128 * (din + 1)],
                            rhs=xt[:, 2048 * din + 512 * qcc : 2048 * din + 512 * (qcc + 1)],
                            start=(din == 0),
                            stop=(din == NDIN - 1),
                        )
                    nc.vector.tensor_scalar_add(
                        dst[:, 512 * (4 * b + qcc) : 512 * (4 * b + qcc + 1)],
                        pt_[:],
                        b_sb[:],
                    )

                return [lambda q=q: chunk_unit(q) for q in range(4)]

            # ============ phase 1a: K+Q projections (batch 0) ============
            with tc.tile_pool(name="ps1a", bufs=4, space="PSUM") as ps1a:
                warm = ps1a.tile([128, 512], F32, name="warm", tag="ps")
                for i in range(32):
                    nc.tensor.matmul(
                        warm[:, 0:128], lhsT=ones_sb[:], rhs=ones_sb[:],
                        start=True, stop=True,
                    )
                for u in qk_proj(ps1a, "k", xk_b[0], wk_sb, bk_sb, kt_sb, 0, 4):
                    u()
                for u in qk_proj(ps1a, "q", xq_b[0], wq_sb, bq_sb, qt_sb, 0, 4):
                    u()

            # =================== phase 2: attention ===================
            # query ownership: core j owns queries [128j, 128j+128) of each
            # 1024-query window w (4 windows); one AllToAll per window, so
            # the first three hide under attention and the last is 128 KB.
            a2a_ins = [dram.tile([1024, 128], FP8, name=f"a2ai{w}") for w in range(4)]
            a2a_outs = [dram.tile([1024, 128], FP8, name=f"a2ao{w}") for w in range(4)]

            pjp = tc.alloc_tile_pool(name="pjp", bufs=4, space="PSUM")  # 4 banks

            def emit_qk(qc, kt):
                b = qc // 4
                q0 = 512 * qc
                kk = 2048 * b + 128 * kt
                st = stp.tile([128, 1024], F32, name=f"st{qc}_{kt}", tag="st")
                pt = ptp.tile([128, 1024], BF16, name=f"pt{qc}_{kt}", tag="pt")
                for h in range(2):
                    nc.tensor.matmul(
                        st[:, 512 * h : 512 * (h + 1)],
                        lhsT=kt_sb[64 * h : 64 * (h + 1), kk : kk + 128],
                        rhs=qt_sb[64 * h : 64 * (h + 1), q0 : q0 + 512],
                        start=True,
                        stop=True,
                    )
                nc.scalar.activation(pt[:], st[:], AF.Exp, scale=0.125 / (WSCALE * WSCALE))
                return pt

            def emit_pv(qc, kt, ot, rs, pt):
                b = qc // 4
                vk = 128 * (16 * b + kt)
                first = kt == 0
                last = kt == NKT - 1
                for h in range(2):
                    nc.tensor.matmul(
                        ot[64 * h : 64 * (h + 1), :],
                        lhsT=v_sb[:, vk + 64 * h : vk + 64 * (h + 1)],
                        rhs=pt[:, 512 * h : 512 * (h + 1)],
                        start=first,
                        stop=last,
                        tile_position=(0, 64 * h),
                    )
                for h in range(2):
                    nc.tensor.matmul(
                        rs[32 * h : 32 * h + 1, :],
                        lhsT=ones_col[:],
                        rhs=pt[:, 512 * h : 512 * (h + 1)],
                        start=first,
                        stop=last,
                        tile_position=(0, 32 * h),
                    )

            def finish(qc, ot_sb, rs_bf):
                # normalize + residual; PE part (bc) rides in the middle of
                # the next round's matmul stream
                q0 = 512 * qc
                bc = rsp.tile([128, 512], F32, name=f"bc{qc}", tag="rs")
                nc.tensor.matmul(
                    bc[0:64, :], lhsT=ones_sb[0:1, 0:64], rhs=rs_bf[0:1, :],
                    start=True, stop=True, tile_position=(0, 0),
                )
                nc.tensor.matmul(
                    bc[64:128, :], lhsT=ones_sb[32:33, 0:64], rhs=rs_bf[32:33, :],
                    start=True, stop=True, tile_position=(32, 64),
                )
                recipb = nrm.tile([128, 512], F32, name=f"recipb{qc}", tag="recipb")
                a2a_in = a2a_ins[qc // 2]
                # halves pipelined: the first two a2a blocks ship while the
                # second half's reciprocal still runs (shaves tail latency)
                for half in range(2):
                    h0 = 256 * half
                    nc.vector.reciprocal(recipb[:, h0 : h0 + 256], bc[:, h0 : h0 + 256])
                    nc.vector.tensor_tensor(
                        zt_local[:, q0 + h0 : q0 + h0 + 256],
                        ot_sb[:, h0 : h0 + 256],
                        recipb[:, h0 : h0 + 256],
                        ALU.mult,
                    )
                    for blk in (2 * half, 2 * half + 1):
                        j = 4 * (qc % 2) + blk
                        nc.sync.dma_start(
                            a2a_in[128 * j : 128 * (j + 1), :],
                            zt_local[:, q0 + 128 * blk : q0 + 128 * (blk + 1)],
                        )

            # prologue: qk(qc0) first (earliest exp), V b0, qk(qc1), then
            # all batch-1 projection units (their DMAs land later).
            ptsd = {}
            ptsd[0] = [emit_qk(0, kt) for kt in range(NKT)]
            for kt in range(NKT):
                v_unit(pjp, 0, kt)
            ptsd[1] = [emit_qk(1, kt) for kt in range(NKT)]
            for u in qk_proj(pjp, "k", xk_b[1], wk_sb, bk_sb, kt_sb, 1, 4):
                u()
            for u in qk_proj(pjp, "q", xq_b[1], wq_sb, bq_sb, qt_sb, 1, 4):
                u()
            for kt in range(NKT):
                v_unit(pjp, 1, kt)
            pjp.release()
            otp = tc.alloc_tile_pool(name="otp", bufs=2, space="PSUM")
            rsp = tc.alloc_tile_pool(name="rsp", bufs=2, space="PSUM")

            pending = None
            for qc in range(NQC):
                ot = otp.tile([128, 512], F32, name=f"ot{qc}", tag="ot")
                rs = rsp.tile([128, 512], F32, name=f"rs{qc}", tag="rs")
                pts = ptsd.pop(qc)
                nxt = []
                for kt in range(NKT):
                    emit_pv(qc, kt, ot, rs, pts[kt])
                    if qc + 2 < NQC:
                        nxt.append(emit_qk(qc + 2, kt))
                    if kt == 0 and pending is not None:
                        finish(*pending)
                        if pending[0] % 2 == 1:
                            w = pending[0] // 2
                            nc.gpsimd.collective_compute(
                                "AllToAll",
                                ALU.bypass,
                                replica_groups=[list(range(N_CORES))],
                                ins=[a2a_ins[w].opt()],
                                outs=[a2a_outs[w].opt()],
                            )
                        pending = None
                if qc + 2 < NQC:
                    ptsd[qc + 2] = nxt
                # drain psum accumulators to SBUF on VectorE so banks free
                # without PE stalls
                ot_sb = nrm.tile([128, 512], F32, name=f"otsb{qc}", tag="otsb")
                nc.vector.tensor_copy(ot_sb[:], ot[:])
                rs_bf = nrm.tile([128, 512], BF16, name=f"rsbf{qc}", tag="rsbf")
                nc.vector.tensor_copy(rs_bf[0:1, :], rs[0:1, :])
                nc.vector.tensor_copy(rs_bf[32:33, :], rs[32:33, :])
                if pending is not None:
                    finish(*pending)
                pending = (qc, ot_sb, rs_bf)
            finish(*pending)

            # last collective fires as soon as its inputs land
            nc.gpsimd.collective_compute(
                "AllToAll",
                ALU.bypass,
                replica_groups=[list(range(N_CORES))],
                ins=[a2a_ins[3].opt()],
                outs=[a2a_outs[3].opt()],
            )

            rsp.release()
            otp.release()
            xin.release()
            xvp.release()

            # =================== phase 3: unpack + fc ===================
            # per-window unpack + residual + fc; first three windows' work
            # overlaps the last AllToAll.  tile_wait_until keeps the
            # scheduler from hoisting phase-3 engine work ahead of the
            # attention drains (head-of-line blocking otherwise).
            with (
                tc.tile_pool(name="fcps", bufs=4, space="PSUM") as fcps,
                tc.tile_pool(name="ysb", bufs=2) as ysb,
                tc.tile_wait_until(0.5),
            ):
                for w in range(4):
                    for t in range(NDIN):
                        nc.sync.dma_start(
                            zt_local[:, 512 * t + 128 * w : 512 * t + 128 * (w + 1)],
                            a2a_outs[w][128 * t : 128 * (t + 1), :],
                        )
                    for t in range(NDIN):
                        c0 = 512 * t + 128 * w
                        nc.vector.scalar_tensor_tensor(
                            qt_sb[:, c0 : c0 + 128],
                            zt_local[:, c0 : c0 + 128],
                            1.0 / WSCALE,
                            xresfc_sb[:, c0 : c0 + 128],
                            ALU.mult,
                            ALU.add,
                        )
                    row = 128 * w
                    y = ysb.tile([128, 1024], F32, name=f"y{w}", tag="y")
                    for nb in range(2):
                        yp = fcps.tile([128, 512], F32, name=f"yp{w}_{nb}", tag="yp")
                        for j in range(NDIN):
                            nc.tensor.matmul(
                                yp[:],
                                lhsT=qt_sb[:, 512 * j + row : 512 * j + row + 128],
                                rhs=wf_sb[:, 1024 * j + 512 * nb : 1024 * j + 512 * (nb + 1)],
                                start=(j == 0),
                                stop=(j == NDIN - 1),
                            )
                        # drain on ScalarE (idle post-attention; DVE still has
                        # the residual adds) and ship each half immediately
                        nc.scalar.copy(y[:, 512 * nb : 512 * (nb + 1)], yp[:])
                        nc.sync.dma_start(
                            out[row : row + 128, 512 * nb : 512 * (nb + 1)],
                            y[:, 512 * nb : 512 * (nb + 1)],
                        )

            nrm.release()
            ptp.release()
            stp.release()

    nc.compile()
    return nc


def _numpy_reference(q_, k_, v_, mask, Wq, bq, Wk, bk, Wv, bv, Wf, bf):
    q_ = np.asarray(q_, np.float32)
    k_ = np.asarray(k_, np.float32)
    v_ = np.asarray(v_, np.float32)
    b = q_.shape[0]

    def split(x):
        return x.reshape(b, -1, H, DH).transpose(0, 2, 1, 3)

    q = split(q_ @ Wq + bq)
    k = split(k_ @ Wk + bk)
    v = split(v_ @ Wv + bv)
    attn = np.einsum("bhqd,bhkd->bhqk", q, k) / np.sqrt(np.float32(DH))
    attn = np.where(np.asarray(mask)[:, None, :, None], attn, np.float32(-1e12))
    attn = attn - attn.max(axis=-1, keepdims=True)
    e = np.exp(attn)
    p = e / e.sum(axis=-1, keepdims=True)
    o = np.einsum("bhqk,bhkd->bhqd", p, v)
    o = o.transpose(0, 2, 1, 3).reshape(b, -1, D)
    return (o + q_) @ Wf + bf


def kernel(q_, k_, v_, mask, Wq, bq, Wk, bk, Wv, bv, Wf, bf):
    mask = np.asarray(mask)
    if not mask.all():
        return _numpy_reference(q_, k_, v_, mask, Wq, bq, Wk, bk, Wv, bv, Wf, bf)

    q_ = np.asarray(q_, np.float32)
    k_ = np.asarray(k_, np.float32)
    v_ = np.asarray(v_, np.float32)

    # transposed, b-major-concatenated inputs (shared across cores)
    xq_f = np.ascontiguousarray(np.concatenate([q_[b].T for b in range(B)], axis=1))
    xq = xq_f.astype(FP8NP)
    xk = np.ascontiguousarray(np.concatenate([k_[b].T for b in range(B)], axis=1)).astype(FP8NP)
    xv = np.ascontiguousarray(np.concatenate([v_[b].T for b in range(B)], axis=1)).astype(FP8NP)
    wf_b = np.ascontiguousarray(
        np.asarray(Wf, np.float32).astype(BF16NP)
        .reshape(8, 128, 1024).transpose(1, 0, 2).reshape(128, 8192)
    )

    def sbufize(a, cols):
        # [1024, cols] -> SBUF layout [128, 8*cols] (din-tile-major columns)
        return np.ascontiguousarray(
            a.reshape(8, 128, cols).transpose(1, 0, 2).reshape(128, 8 * cols)
        )

    in_maps = []
    for c in range(N_CORES):
        d0 = 128 * c
        xres_cols = np.empty((D, 512), np.float32)
        for w in range(4):
            xres_cols[:, 128 * w : 128 * (w + 1)] = q_[w // 2].T[
                :, 1024 * (w % 2) + 128 * c : 1024 * (w % 2) + 128 * (c + 1)
            ]
        in_maps.append(
            {
                "xq": xq,
                "xk": xk,
                "xv": xv,
                "xresfc": sbufize(xres_cols.astype(BF16NP), 512),
                "wq": sbufize((np.asarray(Wq, np.float32)[:, d0 : d0 + 128] * WSCALE).astype(FP8NP), 128),
                "wk": sbufize((np.asarray(Wk, np.float32)[:, d0 : d0 + 128] * WSCALE).astype(FP8NP), 128),
                "wv": sbufize((np.asarray(Wv, np.float32)[:, d0 : d0 + 128] * WSCALE).astype(FP8NP), 128),
                "wf": wf_b,
                "bq": np.ascontiguousarray(np.asarray(bq, np.float32)[d0 : d0 + 128, None] * WSCALE),
                "bk": np.ascontiguousarray(np.asarray(bk, np.float32)[d0 : d0 + 128, None] * WSCALE),
                "bv": np.ascontiguousarray(np.asarray(bv, np.float32)[None, d0 : d0 + 128] * WSCALE).astype(BF16NP),
            }
        )

    if "nc" not in _CACHE:
        _CACHE["nc"] = _build()
    res = run_bass_kernel_spmd(_CACHE["nc"], in_maps, core_ids=list(range(N_CORES)))

    out = np.empty((B, S, D), np.float32)
    for c in range(N_CORES):
        y = res.results[c]["out"]
        for w in range(4):
            out[w // 2, 1024 * (w % 2) + 128 * c : 1024 * (w % 2) + 128 * (c + 1), :] = (
                y[128 * w : 128 * (w + 1)]
            )
    out += np.asarray(bf, np.float32)[None, None, :]
    return out


if __name__ == "__main__":
    rng = np.random.default_rng(0)
    args = dict(
        q_=rng.standard_normal((B, S, D), dtype=np.float32),
        k_=rng.standard_normal((B, S, D), dtype=np.float32),
        v_=rng.standard_normal((B, S, D), dtype=np.float32),
        mask=np.ones((B, S), bool),
        Wq=rng.standard_normal((D, D), dtype=np.float32) * 0.02,
        bq=np.zeros(D, np.float32),
        Wk=rng.standard_normal((D, D), dtype=np.float32) * 0.02,
        bk=np.zeros(D, np.float32),
        Wv=rng.standard_normal((D, D), dtype=np.float32) * 0.02,
        bv=np.zeros(D, np.float32),
        Wf=rng.standard_normal((D, D), dtype=np.float32) * 0.02,
        bf=np.zeros(D, np.float32),
    )
    got = kernel(**args)
    want = _numpy_reference(**args)
    rel = np.abs(got - want).max() / np.abs(want).max()
    print("rel_err:", rel)
